# revision 1
# baseline (speedup 1.0000x reference)
"""Trainium2 Bass kernel for nn_Decoder_36283883716822 (2-layer graph-conv LSTM).

Computation (per reference):
  layer0: supp0 = einsum('knm,bmp->bnkp', G, [x_t|h0]) ; gates0 = supp0 @ W0 + b0
          c_t0 = sig(f)*c0 + sig(i)*tanh(g); h_t0 = sig(o)*tanh(c_t0)
  layer1: same with [h_t0|h1], W1, b1, c1 -> h_t1, c_t1
  returns (h_t1, h_t0, h_t1, c_t0, c_t1)

Strategy: data-parallel over batch B=32 across 8 NeuronCores (4 batches/core).
G (201MB fp32) is streamed from HBM twice per core (once per layer) as the
stationary matmul operand in float32r (full-rate PE path, fp32 bits in memory).
Features for all 4 local batches are stacked along the moving free dim
(N=320 for layer0, N=512 for layer1, both >=256 so f32r runs 1 cycle/row).
supp tiles are PE-transposed so the gates matmul can contract over features.
"""

import sys

sys.path.insert(0, "/opt/trn_rl_repo")

import numpy as np

import concourse.bacc as bacc
import concourse.tile as tile
from concourse import mybir
from concourse.bass_utils import run_bass_kernel_spmd
from concourse.masks import make_identity

F32 = mybir.dt.float32
F32R = mybir.dt.float32r
ACT_COPY = mybir.ActivationFunctionType.Copy
SIGMOID = mybir.ActivationFunctionType.Sigmoid
TANH = mybir.ActivationFunctionType.Tanh

# problem constants
N, B, C, H, K = 4096, 32, 16, 64, 3
NCORES = 8
BL = B // NCORES          # 4 local batches per core
NB = N // 128             # 32 node blocks
P0 = C + H                # 80
P1 = 2 * H                # 128
Q = 4 * H                 # 256
MCH = 16                  # m-blocks per G DMA chunk (1MB)

_CACHE = {}


def _build(loop_iters=None):
    """Build the SPMD kernel. loop_iters wraps the compute in a hardware
    For_i loop (identical re-execution) — used only for timing."""
    nc = bacc.Bacc(trn_type="TRN2")

    gt = nc.dram_tensor("gt", [K, NB, N, 128], F32, kind="ExternalInput")
    comb0_d = nc.dram_tensor("comb0", [NB, 128, BL * P0], F32, kind="ExternalInput")
    h1r_d = nc.dram_tensor("h1r", [NB, 128, BL * H], F32, kind="ExternalInput")
    c0r_d = nc.dram_tensor("c0r", [NB, 128, BL * H], F32, kind="ExternalInput")
    c1r_d = nc.dram_tensor("c1r", [NB, 128, BL * H], F32, kind="ExternalInput")
    w0_d = nc.dram_tensor("w0", [K * P0, Q], F32, kind="ExternalInput")
    w1_d = nc.dram_tensor("w1", [K * P1, Q], F32, kind="ExternalInput")
    b0_d = nc.dram_tensor("b0", [Q], F32, kind="ExternalInput")
    b1_d = nc.dram_tensor("b1", [Q], F32, kind="ExternalInput")
    ht0_d = nc.dram_tensor("ht0", [BL, N, H], F32, kind="ExternalOutput")
    ct0_d = nc.dram_tensor("ct0", [BL, N, H], F32, kind="ExternalOutput")
    ht1_d = nc.dram_tensor("ht1", [BL, N, H], F32, kind="ExternalOutput")
    ct1_d = nc.dram_tensor("ct1", [BL, N, H], F32, kind="ExternalOutput")

    with tile.TileContext(nc) as tc:
        with (
            tc.tile_pool(name="const", bufs=1) as constp,
            tc.tile_pool(name="gt_p", bufs=4) as gt_p,
            tc.tile_pool(name="sup_p", bufs=2) as sup_p,
            tc.tile_pool(name="supT_p", bufs=2) as supT_p,
            tc.tile_pool(name="state_p", bufs=2) as state_p,
            tc.tile_pool(name="lstm_p", bufs=3) as lstm_p,
            tc.tile_pool(name="ps_sup", bufs=1, space="PSUM") as ps_sup,
            tc.tile_pool(name="ps_tr", bufs=2, space="PSUM") as ps_tr,
            tc.tile_pool(name="ps_g", bufs=2, space="PSUM") as ps_g,
        ):
            ident_f32 = constp.tile([128, 128], F32)
            make_identity(nc, ident_f32[:])
            ident = constp.tile([128, 128], F32R)
            nc.vector.tensor_copy(ident[:], ident_f32[:])

            # gate weights, one [P,Q] tile per hop
            w_sb = {}
            for k in range(K):
                t0 = constp.tile([P0, Q], F32R, name=f"w0sb{k}", tag=f"w0sb{k}")
                nc.sync.dma_start(t0[:], w0_d[k * P0:(k + 1) * P0, :].bitcast(F32R))
                w_sb[(0, k)] = t0
                t1 = constp.tile([P1, Q], F32R, name=f"w1sb{k}", tag=f"w1sb{k}")
                nc.sync.dma_start(t1[:], w1_d[k * P1:(k + 1) * P1, :].bitcast(F32R))
                w_sb[(1, k)] = t1
            bias_sb = {}
            for li, bd in ((0, b0_d), (1, b1_d)):
                bt = constp.tile([128, Q], F32, name=f"bias{li}", tag=f"bias{li}")
                nc.sync.dma_start(
                    bt[:], bd[:].rearrange("(o q) -> o q", o=1).to_broadcast((128, Q))
                )
                bias_sb[li] = bt

            # resident feature tiles: comb0 [128, NB*320], comb1 [128, NB*512]
            comb0_sb = constp.tile([128, NB * BL * P0], F32R)
            nc.sync.dma_start(
                comb0_sb[:].rearrange("p (j f) -> p j f", f=BL * P0),
                comb0_d[:].rearrange("j p f -> p j f").bitcast(F32R),
            )
            comb1_sb = constp.tile([128, NB * BL * P1], F32R)
            # pre-fill the h1 half: comb1[p, j, b, 64:128] = h1r[j, p, b*64:+64]
            for j in range(NB):
                nc.sync.dma_start(
                    comb1_sb[:, j * BL * P1:(j + 1) * BL * P1]
                    .rearrange("p (b t f) -> p b t f", t=2, f=H)[:, :, 1, :],
                    h1r_d[j].rearrange("p (b f) -> p b f", b=BL).bitcast(F32R),
                )

            def emit_compute():
                emit_layers(
                    nc, tc, gt, comb0_sb, comb1_sb, w_sb, bias_sb, ident,
                    c0r_d, c1r_d, ht0_d, ct0_d, ht1_d, ct1_d,
                    gt_p, sup_p, supT_p, state_p, lstm_p, ps_sup, ps_tr, ps_g,
                )

            if loop_iters is None:
                emit_compute()
            else:
                with tc.For_i(0, loop_iters, 1):
                    emit_compute()

    nc.compile()
    return nc


def emit_layers(
    nc, tc, gt, comb0_sb, comb1_sb, w_sb, bias_sb, ident,
    c0r_d, c1r_d, ht0_d, ct0_d, ht1_d, ct1_d,
    gt_p, sup_p, supT_p, state_p, lstm_p, ps_sup, ps_tr, ps_g,
):
    if True:
        if True:
            for li in range(2):
                P = P0 if li == 0 else P1
                comb_sb = comb0_sb if li == 0 else comb1_sb
                cpre_d = c0r_d if li == 0 else c1r_d
                ht_d = ht0_d if li == 0 else ht1_d
                ct_d = ct0_d if li == 0 else ct1_d
                W = BL * P  # moving width of the supp matmul

                for nj in range(NB):
                    psk = [
                        ps_sup.tile([128, W], F32, name=f"psk{k}", tag=f"psk{k}")
                        for k in range(K)
                    ]
                    for k in range(K):
                        for mc in range(NB // MCH):
                            gtt = gt_p.tile([128, MCH * 128], F32R, tag="gt", name="gtt")
                            # alternate between the two HWDGE queues (SP / ACT)
                            dma_eng = nc.sync if (k * (NB // MCH) + mc) % 2 == 0 else nc.scalar
                            dma_eng.dma_start(
                                gtt[:].rearrange("p (j c) -> p j c", c=128),
                                gt[k, nj, mc * MCH * 128:(mc + 1) * MCH * 128, :]
                                .rearrange("(j p) c -> p j c", p=128)
                                .bitcast(F32R),
                            )
                            for j in range(MCH):
                                mj = mc * MCH + j
                                nc.tensor.matmul(
                                    psk[k][:],
                                    gtt[:, j * 128:(j + 1) * 128],
                                    comb_sb[:, mj * W:(mj + 1) * W],
                                    start=(mj == 0),
                                    stop=(mj == NB - 1),
                                )
                    sup_sb = [
                        sup_p.tile([128, W], F32R, name=f"sup{k}", tag=f"sup{k}")
                        for k in range(K)
                    ]
                    for k in range(K):
                        nc.scalar.activation(sup_sb[k][:], psk[k][:], ACT_COPY)
                    supT = {}
                    for b in range(BL):
                        for k in range(K):
                            ptr = ps_tr.tile([P, 128], F32R, tag="tr", name="ptr")
                            nc.tensor.transpose(
                                ptr[:], sup_sb[k][:, b * P:(b + 1) * P], ident[:]
                            )
                            st = supT_p.tile(
                                [P, 128], F32R, tag=f"sT{b}_{k}", name=f"sT{b}_{k}"
                            )
                            nc.vector.tensor_copy(st[:], ptr[:])
                            supT[(b, k)] = st

                    cpre = state_p.tile([128, BL * H], F32, tag="cpre", name="cpre")
                    nc.gpsimd.dma_start(cpre[:], cpre_d[nj])

                    for b in range(BL):
                        psg = ps_g.tile([128, Q], F32, tag="g", name="psg")
                        for k in range(K):
                            nc.tensor.matmul(
                                psg[:],
                                supT[(b, k)][:],
                                w_sb[(li, k)][:],
                                start=(k == 0),
                                stop=(k == K - 1),
                            )
                        nc.vector.tensor_add(psg[:], psg[:], bias_sb[li][:])
                        sig_ifo = lstm_p.tile([128, 3 * H], F32, tag="sig", name="sig_ifo")
                        nc.scalar.activation(sig_ifo[:], psg[:, 0:3 * H], SIGMOID)
                        tanh_g = lstm_p.tile([128, H], F32, tag="tg", name="tanh_g")
                        nc.scalar.activation(tanh_g[:], psg[:, 3 * H:4 * H], TANH)
                        ct_t = lstm_p.tile([128, H], F32, tag="ct", name="ct_t")
                        nc.vector.tensor_mul(
                            ct_t[:], sig_ifo[:, H:2 * H], cpre[:, b * H:(b + 1) * H]
                        )
                        ig = lstm_p.tile([128, H], F32, tag="ig", name="ig")
                        nc.vector.tensor_mul(ig[:], sig_ifo[:, 0:H], tanh_g[:])
                        nc.vector.tensor_add(ct_t[:], ct_t[:], ig[:])
                        tanh_c = lstm_p.tile([128, H], F32, tag="tc", name="tanh_c")
                        nc.scalar.activation(tanh_c[:], ct_t[:], TANH)
                        ht_t = lstm_p.tile([128, H], F32, tag="ht", name="ht_t")
                        nc.vector.tensor_mul(ht_t[:], sig_ifo[:, 2 * H:3 * H], tanh_c[:])
                        if li == 0:
                            # feed layer1: comb1[p, nj, b, 0:64] = h_t0
                            nc.scalar.activation(
                                comb1_sb[:, (nj * BL + b) * P1:(nj * BL + b) * P1 + H],
                                ht_t[:],
                                ACT_COPY,
                            )
                        nc.gpsimd.dma_start(ct_d[b, nj * 128:(nj + 1) * 128, :], ct_t[:])
                        nc.gpsimd.dma_start(ht_d[b, nj * 128:(nj + 1) * 128, :], ht_t[:])


def _prep_inputs(G, x_t, h0, h1, c0, c1, W0, b0, W1, b1):
    """Host-side reshapes into the DMA-friendly layouts (shared + per-core)."""
    G = np.ascontiguousarray(G, dtype=np.float32)
    # gt[k, nj, m, c] = G[k, nj*128+c, m]
    gtc = np.ascontiguousarray(
        G.reshape(K, NB, 128, N).transpose(0, 1, 3, 2)
    )
    comb_full = np.concatenate(
        [np.asarray(x_t, np.float32), np.asarray(h0, np.float32)], axis=-1
    )  # [B, N, 80]

    def per_core_pack(arr, F):
        # [B, N, F] -> per core [NB, 128, BL*F]
        out = []
        for ci in range(NCORES):
            a = arr[ci * BL:(ci + 1) * BL].reshape(BL, NB, 128, F)
            out.append(np.ascontiguousarray(a.transpose(1, 2, 0, 3)).reshape(NB, 128, BL * F))
        return out

    comb0_pc = per_core_pack(comb_full, P0)
    h1_pc = per_core_pack(np.asarray(h1, np.float32), H)
    c0_pc = per_core_pack(np.asarray(c0, np.float32), H)
    c1_pc = per_core_pack(np.asarray(c1, np.float32), H)
    w0 = np.ascontiguousarray(W0, np.float32)
    w1 = np.ascontiguousarray(W1, np.float32)
    b0 = np.ascontiguousarray(b0, np.float32)
    b1 = np.ascontiguousarray(b1, np.float32)

    in_maps = []
    for ci in range(NCORES):
        in_maps.append(
            {
                "gt": gtc,
                "comb0": comb0_pc[ci],
                "h1r": h1_pc[ci],
                "c0r": c0_pc[ci],
                "c1r": c1_pc[ci],
                "w0": w0,
                "w1": w1,
                "b0": b0,
                "b1": b1,
            }
        )
    return in_maps


def _assemble(results):
    def cat(name):
        return np.concatenate([results[ci][name] for ci in range(NCORES)], axis=0)

    ht0 = cat("ht0")
    ct0 = cat("ct0")
    ht1 = cat("ht1")
    ct1 = cat("ct1")
    return (ht1, ht0, ht1, ct0, ct1)


def kernel(G, x_t, h0, h1, c0, c1, W0, b0, W1, b1):
    if "nc" not in _CACHE:
        _CACHE["nc"] = _build()
    nc = _CACHE["nc"]
    in_maps = _prep_inputs(G, x_t, h0, h1, c0, c1, W0, b0, W1, b1)
    res = run_bass_kernel_spmd(nc, in_maps, core_ids=list(range(NCORES)))
    return _assemble(res.results)

